# revision 69
# baseline (speedup 1.0000x reference)
"""Trainium2 Bass kernel for AdultConnectomeNetwork (gnn_message_passing).

Reference computation:
    A = scatter(rows, cols, adj_vals)   # [N, N] dense from COO, dups sum
    W = scatter(rows, cols, w_vals)     # [N, N]
    M = A @ W                           # nearly dense (avg degree 64)
    for _ in range(4): x = M @ x + bias[None, :]

Distribution (8 NeuronCores): column-shard x. Core c owns x[:, c*256:(c+1)*256].
  Phase 1: core c computes McT = (A @ W[:, cblk]).T = W[:, cblk].T @ A.T
           (lhsT = Wc tiles, rhs = AT streamed from DRAM)  -> [256, 2048]
  Phase 2: AllGather over the partition axis -> full MT = M.T on every core.
           Pipelined in CHUNKS column blocks: AG of chunk h overlaps
           phase-1 of chunk h+1 and the layer-0 front.
  Phase 3: 4 layers of x_blk = M @ x_blk + bias_blk, fully SBUF-resident,
           no further communication. lhsT = MT tiles, rhs = x k-tiles.
           bias is added on DVE during the PSUM->SBUF copy of each tile.
Host: dense scatter of the COO (np.bincount), shard, run SPMD, concat cols.
"""

import numpy as np

import concourse.bass as bass
import concourse.mybir as mybir
from concourse import bacc, tile
from concourse.bass_utils import run_bass_kernel_spmd

N = 2048
NNZ = 131072
LAYERS = 4
N_CORES = 8
NB = N // N_CORES          # 256 columns of x per core
KT = N // 128              # 16 k-tiles
PH1_N = 512                # phase-1 moving width cap per matmul
# AG pipeline chunk widths (MT column blocks, sum = N). Small first chunk
# starts the AG pipeline early; small last chunk shortens the tail chain
# (AG + SBUF load + layer-0 quarter of the final chunk gate the layers).
CHUNK_WS = [448, 448, 384, 384, 384]
CHUNKS = len(CHUNK_WS)
CHUNK_OFF = [sum(CHUNK_WS[:i]) for i in range(CHUNKS + 1)]

# matmul compute dtype: float32r = full-rate fp32 mode (1 cyc/row at N>=256,
# ~1e-4 rel err end to end). bfloat16 halves DMA/AG bytes (~5e-3 rel err).
DEFAULT_DT = "bf16"
_DT = {"f32r": mybir.dt.float32r, "f32": mybir.dt.float32, "bf16": mybir.dt.bfloat16}


def build_nc(iters: int = 1, sim_single_core: bool = False, dt: str = DEFAULT_DT) -> bacc.Bacc:
    """sim_single_core: replace each AllGather with 8 slice-copy DMAs so the
    graph is collective-free (runnable under TimelineSim) while keeping the
    same dependency structure. That variant is NOT functionally correct."""
    st = _DT[dt]
    nc = bacc.Bacc("TRN2", target_bir_lowering=False, num_devices=N_CORES)
    f32 = mybir.dt.float32

    at_d = nc.dram_tensor("at", [N, N], st, kind="ExternalInput")
    wc_d = nc.dram_tensor("wc", [N, NB], st, kind="ExternalInput")
    xc_d = nc.dram_tensor("xc", [N, NB], st, kind="ExternalInput")
    # bias block replicated across partitions on host; added on DVE during
    # the PSUM->SBUF copy of each layer's output tile
    bias_d = nc.dram_tensor("biasc", [128, NB], f32, kind="ExternalInput")
    out_d = nc.dram_tensor("out", [N, NB], f32, kind="ExternalOutput")

    with tile.TileContext(nc) as tc:
        with (
            tc.tile_pool(name="const", bufs=1) as constp,
            tc.tile_pool(name="mt", bufs=1) as mtp,
            tc.tile_pool(name="x", bufs=1) as xp,
            tc.tile_pool(name="dram", bufs=1, space="DRAM") as dram,
        ):
            # persistent SBUF tensors
            mt_sb = mtp.tile([128, KT * N], st, tag="mt")       # full MT
            xa = xp.tile([128, KT * NB], st, tag="xa")
            xb = xp.tile([128, KT * NB], st, tag="xb")
            bias_sb = constp.tile([128, NB], f32, tag="bias")

            for it in range(iters):
                # AG bounce buffers, one per McT column chunk (fresh per
                # iteration: Shared DRAM allows only a single writer)
                mct_h = [dram.tile([NB, CHUNK_WS[h]], st, name=f"mct_h{h}_{it}")
                         for h in range(CHUNKS)]
                # Shared addr space for collective outputs (fast HBM-HBM
                # path); the sim twin has 8 slice-writers, which Shared
                # forbids
                mt_as = "Local" if sim_single_core else "Shared"
                mt_h = [dram.tile([N, CHUNK_WS[h]], st, name=f"mt_h{h}_{it}",
                                  addr_space=mt_as)
                        for h in range(CHUNKS)]
                # ---- Phase 1: McT = WcT @ AT ----
                with (
                    tc.tile_pool(name="wc", bufs=1) as wcp,
                    tc.tile_pool(name="at", bufs=4) as atp,
                    tc.tile_pool(name="ps1", bufs=3, space="PSUM") as ps1p,
                    tc.tile_pool(name="mcts", bufs=4) as mcp,
                ):
                    # wc on the scalar HWDGE ring so the sync ring is free
                    # for the AT stream (two independent FIFO rings)
                    wc_sb = wcp.tile([128, KT * NB], st, tag="wc")
                    wc_3d = wc_sb[:, :].rearrange("p (k c) -> p k c", k=KT)
                    wc_src = wc_d[:, :].rearrange("(k p) c -> p k c", p=128)
                    for q in range(4):
                        # q0 on the sync ring: lands before the AT stream's
                        # first piece, so the first matmul starts earlier
                        eng = nc.sync if q == 0 else nc.scalar
                        eng.dma_start(
                            out=wc_3d[:, 4 * q:4 * (q + 1), :],
                            in_=wc_src[:, 4 * q:4 * (q + 1), :],
                        )

                    def phase1_half(h):
                        # decompose the chunk into <=512-wide n-segments
                        segs = []
                        off = 0
                        while off < CHUNK_WS[h]:
                            w = min(PH1_N, CHUNK_WS[h] - off)
                            segs.append((off, w))
                            off += w
                        for nn, (soff, w) in enumerate(segs):
                            col0 = CHUNK_OFF[h] + soff
                            pss = [
                                ps1p.tile([128, PH1_N], f32, name=f"ps1_{mi}", tag="ps1t")
                                for mi in range(2)
                            ]
                            for kh in range(2):
                                at_t = atp.tile([128, KT // 2, PH1_N], st, tag="at", name="at_t")
                                at_src = at_d[1024 * kh:1024 * (kh + 1),
                                              col0:col0 + w] \
                                    .rearrange("(k p) c -> p k c", p=128)
                                if h == 0 and nn == 0 and kh == 0:
                                    # split the very first transfer so the
                                    # first matmul starts ~2us earlier
                                    nc.sync.dma_start(out=at_t[:, 0:2, :w],
                                                      in_=at_src[:, 0:2, :])
                                    nc.sync.dma_start(out=at_t[:, 2:4, :w],
                                                      in_=at_src[:, 2:4, :])
                                    nc.sync.dma_start(out=at_t[:, 4:, :w],
                                                      in_=at_src[:, 4:, :])
                                else:
                                    nc.sync.dma_start(out=at_t[:, :, :w], in_=at_src)
                                for kk in range(KT // 2):
                                    k = kh * (KT // 2) + kk
                                    for mi in range(2):
                                        nc.tensor.matmul(
                                            pss[mi][:, :w],
                                            wc_sb[:, NB * k + 128 * mi: NB * k + 128 * (mi + 1)],
                                            at_t[:, kk, :w],
                                            start=(k == 0),
                                            stop=(k == KT - 1),
                                        )
                            for mi in range(2):
                                mct_sb = mcp.tile([128, PH1_N], st, tag="mct", name="mct_sb")
                                nc.vector.tensor_copy(mct_sb[:, :w], pss[mi][:, :w])
                                nc.sync.dma_start(
                                    out=mct_h[h][128 * mi:128 * (mi + 1),
                                                 soff:soff + w],
                                    in_=mct_sb[:, :w],
                                )

                    def allgather_half(h):
                        if sim_single_core:
                            # dependency-equivalent stand-in on the SWDGE path
                            # (the real collective runs on separate silicon)
                            for r in range(N_CORES):
                                nc.gpsimd.dma_start(
                                    out=mt_h[h][NB * r:NB * (r + 1), :],
                                    in_=mct_h[h][:, :],
                                )
                        else:
                            nc.gpsimd.collective_compute(
                                "AllGather",
                                mybir.AluOpType.bypass,
                                replica_groups=[list(range(N_CORES))],
                                ins=[mct_h[h].opt()],
                                outs=[mt_h[h].opt()],
                            )

                    mt_sb_3d = mt_sb[:, :].rearrange("p (k c) -> p k c", k=KT)

                    def load_mt_half(h):
                        # k-tile block [:, k*N:(k+1)*N] holds MT[128k.., all m];
                        # chunk h occupies cols [CHUNK_OFF[h], CHUNK_OFF[h+1])
                        src3d = mt_h[h][:, :].rearrange("(k p) c -> p k c", p=128)
                        npieces = 8
                        kstep = KT // npieces
                        for kq in range(npieces):
                            nc.scalar.dma_start(
                                out=mt_sb_3d[:, kstep * kq:kstep * (kq + 1),
                                             CHUNK_OFF[h]:CHUNK_OFF[h + 1]],
                                in_=src3d[:, kstep * kq:kstep * (kq + 1), :],
                            )

                    # x / bias early: tiny transfers, keep the scalar ring
                    # dependency-monotone (each chunk's mct -> AG -> load)
                    if it == 0:
                        nc.scalar.dma_start(out=bias_sb[:, :], in_=bias_d[:, :])
                    nc.scalar.dma_start(
                        out=xa[:, :].rearrange("p (k c) -> p k c", k=KT),
                        in_=xc_d[:, :].rearrange("(k p) c -> p k c", p=128),
                    )
                    for h in range(CHUNKS):
                        phase1_half(h)
                        allgather_half(h)
                        load_mt_half(h)

                # ---- Phase 3: 4 propagation layers ----
                with (
                    tc.tile_pool(name="ps3", bufs=5, space="PSUM") as ps3p,
                    tc.tile_pool(name="xo", bufs=4) as xop,
                ):
                    for layer in range(LAYERS):
                        src = xa if layer % 2 == 0 else xb
                        dst = xb if layer % 2 == 0 else xa
                        last = layer == LAYERS - 1
                        for m in range(KT):
                            ps = ps3p.tile([128, NB], f32, tag="ps", name="ps")
                            for k in range(KT):
                                nc.tensor.matmul(
                                    ps[:, :],
                                    mt_sb[:, N * k + 128 * m: N * k + 128 * (m + 1)],
                                    src[:, NB * k:NB * (k + 1)],
                                    start=(k == 0),
                                    stop=(k == KT - 1),
                                )
                            if last:
                                xo = xop.tile([128, NB], f32, tag="xo", name="xo")
                                nc.vector.tensor_tensor(
                                    xo[:, :], ps[:, :], bias_sb[:, :],
                                    mybir.AluOpType.add,
                                )
                                nc.sync.dma_start(
                                    out=out_d[128 * m:128 * (m + 1), :],
                                    in_=xo[:, :],
                                )
                            else:
                                nc.vector.tensor_tensor(
                                    dst[:, NB * m:NB * (m + 1)], ps[:, :], bias_sb[:, :],
                                    mybir.AluOpType.add,
                                )

    nc.compile()
    return nc


def make_in_maps(x, rows, cols, adj_vals, w_vals, bias, dt: str = DEFAULT_DT):
    """Host-side scatter + shard. Returns in_maps for cores 0..7."""
    rows = np.asarray(rows).astype(np.int64)
    cols = np.asarray(cols).astype(np.int64)
    adj_vals = np.asarray(adj_vals, dtype=np.float64)
    w_vals = np.asarray(w_vals, dtype=np.float64)
    x = np.asarray(x, dtype=np.float32)
    bias = np.asarray(bias, dtype=np.float32)

    # AT[c, r] = sum adj_vals at (r, c)  (A transposed, dense)
    at = np.bincount(cols * N + rows, weights=adj_vals, minlength=N * N).reshape(N, N)
    w = np.bincount(rows * N + cols, weights=w_vals, minlength=N * N).reshape(N, N)

    np_dt = mybir.dt.np(_DT[dt])
    at = np.ascontiguousarray(at, dtype=np_dt)
    w = w.astype(np_dt)
    xs = x.astype(np_dt)

    in_maps = []
    for c in range(N_CORES):
        sl = slice(c * NB, (c + 1) * NB)
        in_maps.append({
            "at": at,
            "wc": np.ascontiguousarray(w[:, sl]),
            "xc": np.ascontiguousarray(xs[:, sl]),
            "biasc": np.ascontiguousarray(
                np.broadcast_to(bias[sl].astype(np.float32), (128, NB))
            ),
        })
    return in_maps


_NC_CACHE = {}


def kernel(x, rows, cols, adj_vals, w_vals, bias):
    if "nc" not in _NC_CACHE:
        _NC_CACHE["nc"] = build_nc(iters=1)
    nc = _NC_CACHE["nc"]
    in_maps = make_in_maps(x, rows, cols, adj_vals, w_vals, bias)
    for attempt in range(2):
        res = run_bass_kernel_spmd(nc, in_maps, core_ids=list(range(N_CORES)))
        out = np.empty((N, N), dtype=np.float32)
        for c in range(N_CORES):
            out[:, c * NB:(c + 1) * NB] = res.results[c]["out"]
        # guard against rare backend transients (observed once): retry on
        # non-finite output; the NEFF itself is deterministic
        if np.isfinite(out).all():
            break
    return out

